# revision 12
# baseline (speedup 1.0000x reference)
"""Trainium2 Bass kernel for nn_ConfidenceLossV2 (segment_reduce, memory-bound).

Sharding: data-parallel over the batch dim — 8 batch items, one per NeuronCore.
Each core computes 4 partial scalars (segment-loss numerator/denominator and
recovery-loss numerator/denominator); the host sums them across cores and does
the two final divisions (the "psum of weighted sums and counts").

Per-core pipeline:
  - enc/dec are loaded with f32->bf16 casting SWDGE DMAs in their NATURAL
    memory order into [128, 8192] tiles where partition k=(c*2+h) holds a
    contiguous 32KB DRAM run (pixel half h of channel c) — every HBM read is
    fully contiguous.  x = enc-dec (DVE bf16 2x) and y = x^2 (ACT) in 4
    pixel-quarter chunks, pipelined against the DMA stream.
  - reco (per-pixel channel mean*64) = fold2.T @ y via PE matmuls (K=128
    contracts all 64 channels for both halves at once), psum -> SBUF, then a
    tiny SBUF->SBUF DMA relayout of the 64KB reco image into pixel-major
    rp[:, 0, :] ([128 r, 128 q] grid); rp block 1 = ones, block 2 = pos.
  - one-hot eq[p, q*64+s] = (seg[p,q]==s) via one DVE is_equal of the
    broadcast seg row against a small host-provided iota row.
  - segment sums: pixel columns PAIRED: lhsT = eq[:, 2t*64 : 2t*64+128]
    (contiguous 128 one-hot weight columns -> FWL), rhs = rp[:, :, 2t:2t+2]
    ([3 cols x 2]), accumulated into psumA [128, 6]; rows 0:64 hold column
    2t's sums, rows 64:128 column 2t+1's; a foldmat matmul adds the halves.
  - recovery loss: per channel-half t=in*lt, d=out-t, dm=d*m01 on DVE, then
    ACT Square with accum_out -> per-partition sums; lt=(mask<0.5) and
    m01=(mask>0) via tensor_scalar, the latter with accum_out giving sum(m)
    for free; pos is just lt[:, ::4]*m01[:, ::4].
  - per-segment selection on [64,1] vectors, then one ones-matmul reduces the
    4 per-partition columns to the [1,4] partials output.
"""

import os
import sys

for _p in ("/opt/trn_rl_repo",):
    if _p not in sys.path and os.path.isdir(_p):
        sys.path.insert(0, _p)

import numpy as np

N_CORES = 8
C_IMG, H, W = 3, 512, 512
C_FEAT, HE, WE = 64, 128, 128
N_SEG = 64
NPIX = HE * WE  # 16384
PIX_FREE = (H * W) // 128  # 2048 free elems per partition at image res
HALF = NPIX // 2  # 8192 pixels per half in the (c,h) layout
WALL_COT = 0.5
MIN_FRAC = 0.01
NQ = 4  # pixel-quarter chunks for the enc/dec stream
QP = HALF // NQ  # 2048 free elems per chunk

_CACHE = {}


def _build():
    import concourse.bacc as bacc
    import concourse.bass as bass
    import concourse.tile as tile
    from concourse import mybir

    dt = mybir.dt
    BF = dt.bfloat16
    F32 = dt.float32
    Alu = mybir.AluOpType
    Act = mybir.ActivationFunctionType

    nc = bacc.Bacc("TRN2", target_bir_lowering=False, debug=False,
                   enable_asserts=False, num_devices=N_CORES)

    outputs_d = nc.dram_tensor("outputs", [C_IMG, H, W], F32, kind="ExternalInput").ap()
    inputs_d = nc.dram_tensor("inputs", [C_IMG, H, W], F32, kind="ExternalInput").ap()
    enc_d = nc.dram_tensor("enc1", [C_FEAT, HE, WE], F32, kind="ExternalInput").ap()
    dec_d = nc.dram_tensor("dec1", [C_FEAT, HE, WE], F32, kind="ExternalInput").ap()
    masks_d = nc.dram_tensor("masks", [H, W], F32, kind="ExternalInput").ap()
    segs_d = nc.dram_tensor("segs", [H, W], dt.int32, kind="ExternalInput").ap()
    iota_d = nc.dram_tensor("iota_row", [128, N_SEG], dt.bfloat16,
                            kind="ExternalInput").ap()
    fold2_d = nc.dram_tensor("fold2", [128, 2], dt.bfloat16,
                             kind="ExternalInput").ap()
    foldm_d = nc.dram_tensor("foldmat", [128, N_SEG], F32,
                             kind="ExternalInput").ap()
    part_d = nc.dram_tensor("partials", [1, 4], F32, kind="ExternalOutput").ap()

    def sub_ap(t, extra_off, dims):
        # manual AP view of a tile: dims = [[step, count], ...] free dims
        return bass.AP(tensor=t.tensor, offset=t.offset + extra_off,
                       ap=[list(t.ap[0])] + [list(d) for d in dims])

    # enc/dec natural-order view: partition k = c*2 + h (contiguous 8192 f32
    # per partition), free = pixel-in-half; sliced into NQ quarter chunks.
    def natview(x, m):
        flat = x.rearrange("c (hh rr) q -> (c hh) (rr q)", hh=2)
        return flat[:, m * QP:(m + 1) * QP]

    with tile.TileContext(nc) as tc:
        with (
            tc.tile_pool(name="big", bufs=1) as big,
            tc.tile_pool(name="xpool", bufs=3) as xpool,
            tc.tile_pool(name="scr", bufs=3) as scrp,
            tc.tile_pool(name="small", bufs=1) as small,
            tc.tile_pool(name="ps", bufs=1, space="PSUM") as psp,
            tc.tile_pool(name="psr", bufs=1, space="PSUM") as psrp,
        ):
            # ---- tiles ----
            seg_rows = big.tile([128, W], dt.int32)        # every 4th image row
            segf = small.tile([128, WE], BF)               # seg ids at feature res
            iota_row = small.tile([128, N_SEG], BF)        # 0..63 per partition
            fold2 = small.tile([128, 2], BF)               # channel-fold ones
            foldm = small.tile([128, N_SEG], F32)          # [I64; I64]
            eq = big.tile([128, WE, N_SEG], BF)            # one-hot, f = q*64+s
            E2 = big.tile([128, HALF], BF)                 # enc, (c,h) layout
            D2 = big.tile([128, HALF], BF)
            ysq = big.tile([128, HALF], BF)                # (enc-dec)^2
            reco_sb = small.tile([2, HALF], BF)            # per-pixel chan sums
            rp = big.tile([128, 3, WE], BF)                # reco | ones | pos
            Mt = big.tile([128, PIX_FREE], BF)
            OT = big.tile([128, C_IMG, PIX_FREE], BF)
            IT = big.tile([128, C_IMG, PIX_FREE], BF)
            lt = big.tile([128, PIX_FREE], BF)
            m01 = big.tile([128, PIX_FREE], BF)
            racc = small.tile([128, 8], F32)
            rhsf = small.tile([128, 4], F32)
            ones128 = small.tile([128, 1], F32)
            sbA = small.tile([128, 6], F32)
            out_sb = small.tile([1, 4], F32)

            err_sum = small.tile([N_SEG, 1], F32)
            counts = small.tile([N_SEG, 1], F32)
            poscnt = small.tile([N_SEG, 1], F32)
            safe = small.tile([N_SEG, 1], F32)
            rsafe = small.tile([N_SEG, 1], F32)
            mean_err = small.tile([N_SEG, 1], F32)
            ratio = small.tile([N_SEG, 1], F32)
            validt = small.tile([N_SEG, 1], F32)
            pflag = small.tile([N_SEG, 1], F32)

            psumA = psp.tile([128, 6], F32)
            psumB = psp.tile([N_SEG, 6], F32)
            psumF = psp.tile([1, 4], F32)

            # ---- DMAs ----
            # cheap deps on the sync queue
            nc.sync.dma_start(out=seg_rows,
                              in_=segs_d.rearrange("(p r) w -> p r w", r=4)[:, 0, :])
            nc.sync.dma_start(out=iota_row, in_=iota_d)
            nc.sync.dma_start(out=fold2, in_=fold2_d)
            nc.sync.dma_start(out=foldm, in_=foldm_d)
            # SWDGE (casting) stream: masks, then enc/dec interleaved by pixel
            # quarter, then outputs/inputs per channel.  One queue = explicit
            # priority order; every HBM read is contiguous.
            nc.gpsimd.dma_start(out=Mt, in_=masks_d.rearrange("(p r) w -> p (r w)", r=4))
            nc.gpsimd.dma_start(
                out=OT, in_=outputs_d.rearrange("c (p r) w -> p c (r w)", r=4))
            for c in range(C_IMG):
                nc.gpsimd.dma_start(
                    out=IT[:, c, :],
                    in_=inputs_d[c].rearrange("(p r) w -> p (r w)", r=4))
            for m in range(NQ):
                nc.gpsimd.dma_start(out=E2[:, m * QP:(m + 1) * QP],
                                    in_=natview(enc_d, m))
                nc.gpsimd.dma_start(out=D2[:, m * QP:(m + 1) * QP],
                                    in_=natview(dec_d, m))

            # ---- one-hot build ----
            # segf = bf16(seg_rows[:, ::4])
            nc.vector.tensor_copy(out=segf, in_=sub_ap(seg_rows, 0, [[4, WE]]))
            # eq[p, q, s] = (segf[p, q] == iota[s])
            segf_b = sub_ap(segf, 0, [[1, WE], [0, N_SEG]])
            iota_b = sub_ap(iota_row, 0, [[0, WE], [1, N_SEG]])
            nc.vector.tensor_tensor(out=eq, in0=segf_b, in1=iota_b, op=Alu.is_equal)

            # ---- mask-derived tensors ----
            nc.vector.memset(rhsf, 0.0)
            nc.vector.tensor_scalar(out=lt, in0=Mt, scalar1=WALL_COT, scalar2=None,
                                    op0=Alu.is_lt)
            # op1 is the accumulation op when accum_out is given
            nc.vector.tensor_scalar(out=m01, in0=Mt, scalar1=0.0, scalar2=None,
                                    op0=Alu.is_gt, op1=Alu.add,
                                    accum_out=rhsf[:, 3:4])
            nc.vector.memset(rp[:, 1, :], 1.0)
            # pos = lt[:, ::4] * m01[:, ::4]  (= (mask_i<0.5)&(mask_i>0))
            nc.vector.tensor_tensor(out=rp[:, 2, :],
                                    in0=sub_ap(lt, 0, [[4, WE]]),
                                    in1=sub_ap(m01, 0, [[4, WE]]), op=Alu.mult)

            # ---- recovery loss elementwise (channel halves for pipelining) ----
            HP = PIX_FREE // 2
            for c in range(C_IMG):
                for h in range(2):
                    osl = sub_ap(OT, c * PIX_FREE + h * HP, [[1, HP]])
                    isl = sub_ap(IT, c * PIX_FREE + h * HP, [[1, HP]])
                    ltl = sub_ap(lt, h * HP, [[1, HP]])
                    ml = sub_ap(m01, h * HP, [[1, HP]])
                    ttile = scrp.tile([128, HP], BF, tag="t")
                    dtile = scrp.tile([128, HP], BF, tag="d")
                    dmt = scrp.tile([128, HP], BF, tag="dm")
                    sq = scrp.tile([128, HP], BF, tag="sq")
                    nc.vector.tensor_tensor(out=ttile, in0=isl, in1=ltl, op=Alu.mult)
                    nc.vector.tensor_tensor(out=dtile, in0=osl, in1=ttile,
                                            op=Alu.subtract)
                    nc.vector.tensor_tensor(out=dmt, in0=dtile, in1=ml, op=Alu.mult)
                    nc.scalar.activation(out=sq, in_=dmt, func=Act.Square,
                                         accum_out=racc[:, 2 * c + h:2 * c + h + 1])
            # sum the 6 per-chunk accumulators into rhsf[:, 2]
            nc.vector.tensor_tensor(out=racc[:, 6:7], in0=racc[:, 0:1],
                                    in1=racc[:, 1:2], op=Alu.add)
            nc.vector.tensor_tensor(out=racc[:, 7:8], in0=racc[:, 2:3],
                                    in1=racc[:, 3:4], op=Alu.add)
            nc.vector.tensor_tensor(out=racc[:, 6:7], in0=racc[:, 6:7],
                                    in1=racc[:, 4:5], op=Alu.add)
            nc.vector.tensor_tensor(out=racc[:, 7:8], in0=racc[:, 7:8],
                                    in1=racc[:, 5:6], op=Alu.add)
            nc.vector.tensor_tensor(out=rhsf[:, 2:3], in0=racc[:, 6:7],
                                    in1=racc[:, 7:8], op=Alu.add)

            # ---- enc/dec -> squares -> reco (chunked against the DMA) ----
            for m in range(NQ):
                sl = slice(m * QP, (m + 1) * QP)
                xg = xpool.tile([128, QP], BF, tag="xg")
                nc.vector.tensor_tensor(out=xg, in0=E2[:, sl], in1=D2[:, sl],
                                        op=Alu.subtract)
                nc.scalar.activation(out=ysq[:, sl], in_=xg, func=Act.Square)
                psumR = psrp.tile([2, QP], F32, tag="psr")
                for i in range(QP // 512):
                    nc.tensor.matmul(psumR[:, i * 512:(i + 1) * 512], fold2,
                                     ysq[:, m * QP + i * 512: m * QP + (i + 1) * 512],
                                     start=True, stop=True)
                nc.scalar.activation(out=reco_sb[:, sl], in_=psumR, func=Act.Copy)
                # relayout this quarter into pixel-major rp[:, 0, :]:
                # half h, pixels p = h*8192 + m*2048 + j -> row 64h+16m+j//128
                for h in range(2):
                    src = reco_sb[h:h + 1, sl].rearrange("p (r q) -> p r q", q=WE)
                    dst = rp[64 * h + 16 * m: 64 * h + 16 * m + QP // WE, 0, :]
                    nc.sync.dma_start(out=dst, in_=src)

            # ---- segment-sum matmuls: paired pixel columns ----
            # lhsT = eq columns [2t*64, 2t*64+128) (contiguous, FWL-friendly),
            # rhs = rp[:, :, 2t:2t+2] -> psumA[128, 6]; rows 64:128 belong to
            # the odd column, cross blocks are garbage and folded away below.
            for t in range(WE // 2):
                lhsT = sub_ap(eq, 2 * t * N_SEG, [[1, 2 * N_SEG]])
                rhs = sub_ap(rp, 2 * t, [[WE, 3], [1, 2]])
                nc.tensor.matmul(psumA, lhsT, rhs, start=(t == 0),
                                 stop=(t == WE // 2 - 1))
            nc.scalar.activation(out=sbA, in_=psumA, func=Act.Copy)
            nc.tensor.matmul(psumB, foldm, sbA, start=True, stop=True)

            # ---- per-segment selection ----
            # psumB cols: reco_q, reco_q1, ones, ones, pos_q, pos_q1
            sbB = small.tile([N_SEG, 6], F32)
            nc.vector.tensor_copy(out=sbB, in_=psumB)
            nc.vector.tensor_tensor(out=err_sum, in0=sbB[:, 0:1],
                                    in1=sbB[:, 1:2], op=Alu.add)
            nc.vector.tensor_tensor(out=counts, in0=sbB[:, 2:3],
                                    in1=sbB[:, 3:4], op=Alu.add)
            nc.vector.tensor_tensor(out=poscnt, in0=sbB[:, 4:5],
                                    in1=sbB[:, 5:6], op=Alu.add)
            nc.vector.tensor_scalar(out=safe, in0=counts, scalar1=1.0, scalar2=None,
                                    op0=Alu.max)
            nc.vector.reciprocal(out=rsafe, in_=safe)
            nc.vector.scalar_tensor_tensor(out=mean_err, in0=err_sum,
                                           scalar=1.0 / C_FEAT, in1=rsafe,
                                           op0=Alu.mult, op1=Alu.mult)
            nc.vector.tensor_tensor(out=ratio, in0=poscnt, in1=rsafe, op=Alu.mult)
            thr_cnt = float(np.float32(MIN_FRAC)) * NPIX
            nc.vector.tensor_scalar(out=validt, in0=counts, scalar1=thr_cnt,
                                    scalar2=None, op0=Alu.is_ge)
            nc.vector.tensor_scalar(out=pflag, in0=ratio,
                                    scalar1=float(np.float32(MIN_FRAC)),
                                    scalar2=None, op0=Alu.is_gt)
            nc.vector.tensor_tensor(out=rhsf[0:N_SEG, 1:2], in0=validt, in1=pflag,
                                    op=Alu.mult)
            nc.vector.tensor_tensor(out=rhsf[0:N_SEG, 0:1], in0=mean_err,
                                    in1=rhsf[0:N_SEG, 1:2], op=Alu.mult)

            # ---- final partition reduction and output ----
            nc.vector.memset(ones128, 1.0)
            nc.tensor.matmul(psumF, ones128, rhsf, start=True, stop=True)
            nc.vector.tensor_copy(out=out_sb, in_=psumF)
            nc.sync.dma_start(out=part_d, in_=out_sb)

    nc.compile()
    return nc


def _get_nc():
    if "nc" not in _CACHE:
        _CACHE["nc"] = _build()
    return _CACHE["nc"]


def _consts():
    import ml_dtypes
    iota = np.ascontiguousarray(
        np.broadcast_to(np.arange(N_SEG, dtype=np.float32), (128, N_SEG))
    ).astype(ml_dtypes.bfloat16)
    f2 = np.zeros((128, 2), dtype=np.float32)
    f2[0::2, 0] = 1.0
    f2[1::2, 1] = 1.0
    fold2 = f2.astype(ml_dtypes.bfloat16)
    fm = np.zeros((128, N_SEG), dtype=np.float32)
    fm[np.arange(64), np.arange(64)] = 1.0
    fm[64 + np.arange(64), np.arange(64)] = 1.0
    return iota, fold2, fm


def kernel(outputs, inputs, enc1, dec1, masks, segs, confidence=None,
           iteration=None, epoch=None, **_unused):
    from concourse import bass_utils

    outputs = np.asarray(outputs, dtype=np.float32)
    inputs = np.asarray(inputs, dtype=np.float32)
    enc1 = np.asarray(enc1, dtype=np.float32)
    dec1 = np.asarray(dec1, dtype=np.float32)
    masks = np.asarray(masks, dtype=np.float32)
    segs = np.asarray(segs, dtype=np.int32)

    nc = _get_nc()
    iota, fold2, foldm = _consts()
    in_maps = []
    for b in range(N_CORES):
        in_maps.append({
            "outputs": np.ascontiguousarray(outputs[b]),
            "inputs": np.ascontiguousarray(inputs[b]),
            "enc1": np.ascontiguousarray(enc1[b]),
            "dec1": np.ascontiguousarray(dec1[b]),
            "masks": np.ascontiguousarray(masks[b, 0]),
            "segs": np.ascontiguousarray(segs[b, 0]),
            "iota_row": iota,
            "fold2": fold2,
            "foldmat": foldm,
        })

    trace = bool(int(os.environ.get("KERNEL_TRACE", "0")))
    res = bass_utils.run_bass_kernel_spmd(nc, in_maps,
                                          core_ids=list(range(N_CORES)),
                                          trace=trace)
    _CACHE["last_result"] = res

    parts = np.stack([res.results[b]["partials"].reshape(4) for b in range(N_CORES)])
    f32 = np.float32
    num1 = f32(parts[:, 0].astype(np.float32).sum(dtype=np.float32))
    den1 = f32(parts[:, 1].astype(np.float32).sum(dtype=np.float32))
    num2 = f32(parts[:, 2].astype(np.float32).sum(dtype=np.float32))
    den2 = f32(parts[:, 3].astype(np.float32).sum(dtype=np.float32))
    flat_pos_mean = f32(num1 / max(den1, f32(1.0)))
    loss_recov = f32(num2 / max(den2, f32(1.0)))
    return np.asarray(f32(loss_recov + flat_pos_mean))


# revision 13
# speedup vs baseline: 1.1676x; 1.1676x over previous
"""Trainium2 Bass kernel for nn_ConfidenceLossV2 (segment_reduce, memory-bound).

Sharding: data-parallel over the batch dim — 8 batch items, one per NeuronCore.
Each core computes 4 partial scalars (segment-loss numerator/denominator and
recovery-loss numerator/denominator); the host sums them across cores and does
the two final divisions (the "psum of weighted sums and counts").

Per-core pipeline:
  - enc/dec are loaded with f32->bf16 casting SWDGE DMAs in their NATURAL
    memory order into [128, 8192] tiles where partition k=(c*2+h) holds a
    contiguous 32KB DRAM run (pixel half h of channel c) — every HBM read is
    fully contiguous.  x = enc-dec (DVE bf16 2x) and y = x^2 (ACT) in 4
    pixel-quarter chunks, pipelined against the DMA stream.
  - reco (per-pixel channel mean*64) = fold2.T @ y via PE matmuls (K=128
    contracts all 64 channels for both halves at once), psum -> SBUF, then a
    tiny SBUF->SBUF DMA relayout of the 64KB reco image into pixel-major
    rp[:, 0, :] ([128 r, 128 q] grid); rp block 1 = ones, block 2 = pos.
  - one-hot eq[p, q*64+s] = (seg[p,q]==s) via one DVE is_equal of the
    broadcast seg row against a small host-provided iota row.
  - segment sums: pixel columns PAIRED: lhsT = eq[:, 2t*64 : 2t*64+128]
    (contiguous 128 one-hot weight columns -> FWL), rhs = rp[:, :, 2t:2t+2]
    ([3 cols x 2]), accumulated into psumA [128, 6]; rows 0:64 hold column
    2t's sums, rows 64:128 column 2t+1's; a foldmat matmul adds the halves.
  - recovery loss: per channel-half t=in*lt, d=out-t, dm=d*m01 on DVE, then
    ACT Square with accum_out -> per-partition sums; lt=(mask<0.5) and
    m01=(mask>0) via tensor_scalar, the latter with accum_out giving sum(m)
    for free; pos is just lt[:, ::4]*m01[:, ::4].
  - per-segment selection on [64,1] vectors, then one ones-matmul reduces the
    4 per-partition columns to the [1,4] partials output.
"""

import os
import sys

for _p in ("/opt/trn_rl_repo",):
    if _p not in sys.path and os.path.isdir(_p):
        sys.path.insert(0, _p)

import numpy as np

N_CORES = 8
C_IMG, H, W = 3, 512, 512
C_FEAT, HE, WE = 64, 128, 128
N_SEG = 64
NPIX = HE * WE  # 16384
PIX_FREE = (H * W) // 128  # 2048 free elems per partition at image res
HALF = NPIX // 2  # 8192 pixels per half in the (c,h) layout
WALL_COT = 0.5
MIN_FRAC = 0.01
NQ = 4  # pixel-quarter chunks for the enc/dec stream
QP = HALF // NQ  # 2048 free elems per chunk

_CACHE = {}


def _build():
    import concourse.bacc as bacc
    import concourse.bass as bass
    import concourse.tile as tile
    from concourse import mybir

    dt = mybir.dt
    BF = dt.bfloat16
    F32 = dt.float32
    Alu = mybir.AluOpType
    Act = mybir.ActivationFunctionType

    nc = bacc.Bacc("TRN2", target_bir_lowering=False, debug=False,
                   enable_asserts=False, num_devices=N_CORES)

    outputs_d = nc.dram_tensor("outputs", [C_IMG, H, W], F32, kind="ExternalInput").ap()
    inputs_d = nc.dram_tensor("inputs", [C_IMG, H, W], F32, kind="ExternalInput").ap()
    enc_d = nc.dram_tensor("enc1", [C_FEAT, HE, WE], F32, kind="ExternalInput").ap()
    dec_d = nc.dram_tensor("dec1", [C_FEAT, HE, WE], F32, kind="ExternalInput").ap()
    masks_d = nc.dram_tensor("masks", [H, W], F32, kind="ExternalInput").ap()
    segs_d = nc.dram_tensor("segs", [H, W], dt.int32, kind="ExternalInput").ap()
    iota_d = nc.dram_tensor("iota_row", [128, N_SEG], dt.bfloat16,
                            kind="ExternalInput").ap()
    fold2_d = nc.dram_tensor("fold2", [128, 2], dt.bfloat16,
                             kind="ExternalInput").ap()
    foldm_d = nc.dram_tensor("foldmat", [128, N_SEG], F32,
                             kind="ExternalInput").ap()
    part_d = nc.dram_tensor("partials", [1, 4], F32, kind="ExternalOutput").ap()

    def sub_ap(t, extra_off, dims):
        # manual AP view of a tile: dims = [[step, count], ...] free dims
        return bass.AP(tensor=t.tensor, offset=t.offset + extra_off,
                       ap=[list(t.ap[0])] + [list(d) for d in dims])

    # enc/dec natural-order view: partition k = c*2 + h (contiguous 8192 f32
    # per partition), free = pixel-in-half; sliced into NQ quarter chunks.
    def natview(x, m):
        flat = x.rearrange("c (hh rr) q -> (c hh) (rr q)", hh=2)
        return flat[:, m * QP:(m + 1) * QP]

    with tile.TileContext(nc) as tc:
        with (
            tc.tile_pool(name="big", bufs=1) as big,
            tc.tile_pool(name="xpool", bufs=3) as xpool,
            tc.tile_pool(name="scr", bufs=3) as scrp,
            tc.tile_pool(name="small", bufs=1) as small,
            tc.tile_pool(name="ps", bufs=1, space="PSUM") as psp,
            tc.tile_pool(name="psr", bufs=1, space="PSUM") as psrp,
        ):
            # ---- tiles ----
            seg_rows = big.tile([128, W], dt.int32)        # every 4th image row
            segf = small.tile([128, WE], BF)               # seg ids at feature res
            iota_row = small.tile([128, N_SEG], BF)        # 0..63 per partition
            fold2 = small.tile([128, 2], BF)               # channel-fold ones
            foldm = small.tile([128, N_SEG], F32)          # [I64; I64]
            eq = big.tile([128, WE, N_SEG], BF)            # one-hot, f = q*64+s
            E2 = big.tile([128, HALF], BF)                 # enc, (c,h) layout
            D2 = big.tile([128, HALF], BF)
            ysq = big.tile([128, HALF], BF)                # (enc-dec)^2
            reco_sb = small.tile([2, HALF], BF)            # per-pixel chan sums
            rp = big.tile([128, 3, WE], BF)                # reco | ones | pos
            Mt = big.tile([128, PIX_FREE], BF)
            OT = big.tile([128, C_IMG, PIX_FREE], BF)
            IT = big.tile([128, C_IMG, PIX_FREE], BF)
            lt = big.tile([128, PIX_FREE], BF)
            m01 = big.tile([128, PIX_FREE], BF)
            racc = small.tile([128, 8], F32)
            rhsf = small.tile([128, 4], F32)
            ones128 = small.tile([128, 1], F32)
            sbA = small.tile([128, 6], F32)
            out_sb = small.tile([1, 4], F32)

            err_sum = small.tile([N_SEG, 1], F32)
            counts = small.tile([N_SEG, 1], F32)
            poscnt = small.tile([N_SEG, 1], F32)
            safe = small.tile([N_SEG, 1], F32)
            rsafe = small.tile([N_SEG, 1], F32)
            mean_err = small.tile([N_SEG, 1], F32)
            ratio = small.tile([N_SEG, 1], F32)
            validt = small.tile([N_SEG, 1], F32)
            pflag = small.tile([N_SEG, 1], F32)

            psumA = psp.tile([128, 6], F32)
            psumB = psp.tile([N_SEG, 6], F32)
            psumF = psp.tile([1, 4], F32)

            # ---- DMAs ----
            # cheap deps on the sync queue
            nc.sync.dma_start(out=seg_rows,
                              in_=segs_d.rearrange("(p r) w -> p r w", r=4)[:, 0, :])
            nc.sync.dma_start(out=iota_row, in_=iota_d)
            nc.sync.dma_start(out=fold2, in_=fold2_d)
            nc.sync.dma_start(out=foldm, in_=foldm_d)
            # SWDGE (casting) stream: masks, then enc/dec interleaved by pixel
            # quarter, then outputs/inputs per channel.  One queue = explicit
            # priority order; every HBM read is contiguous.
            nc.gpsimd.dma_start(out=Mt, in_=masks_d.rearrange("(p r) w -> p (r w)", r=4))
            for m in range(NQ):
                nc.gpsimd.dma_start(out=E2[:, m * QP:(m + 1) * QP],
                                    in_=natview(enc_d, m))
                nc.gpsimd.dma_start(out=D2[:, m * QP:(m + 1) * QP],
                                    in_=natview(dec_d, m))
            for c in range(C_IMG):
                nc.gpsimd.dma_start(
                    out=OT[:, c, :],
                    in_=outputs_d[c].rearrange("(p r) w -> p (r w)", r=4))
                nc.gpsimd.dma_start(
                    out=IT[:, c, :],
                    in_=inputs_d[c].rearrange("(p r) w -> p (r w)", r=4))

            # ---- one-hot build ----
            # segf = bf16(seg_rows[:, ::4])
            nc.vector.tensor_copy(out=segf, in_=sub_ap(seg_rows, 0, [[4, WE]]))
            # eq[p, q, s] = (segf[p, q] == iota[s])
            segf_b = sub_ap(segf, 0, [[1, WE], [0, N_SEG]])
            iota_b = sub_ap(iota_row, 0, [[0, WE], [1, N_SEG]])
            nc.vector.tensor_tensor(out=eq, in0=segf_b, in1=iota_b, op=Alu.is_equal)

            # ---- mask-derived tensors ----
            nc.vector.memset(rhsf, 0.0)
            nc.vector.tensor_scalar(out=lt, in0=Mt, scalar1=WALL_COT, scalar2=None,
                                    op0=Alu.is_lt)
            # op1 is the accumulation op when accum_out is given
            nc.vector.tensor_scalar(out=m01, in0=Mt, scalar1=0.0, scalar2=None,
                                    op0=Alu.is_gt, op1=Alu.add,
                                    accum_out=rhsf[:, 3:4])
            nc.vector.memset(rp[:, 1, :], 1.0)
            # pos = lt[:, ::4] * m01[:, ::4]  (= (mask_i<0.5)&(mask_i>0))
            nc.vector.tensor_tensor(out=rp[:, 2, :],
                                    in0=sub_ap(lt, 0, [[4, WE]]),
                                    in1=sub_ap(m01, 0, [[4, WE]]), op=Alu.mult)

            # ---- enc/dec -> squares -> reco (chunked against the DMA) ----
            for m in range(NQ):
                sl = slice(m * QP, (m + 1) * QP)
                xg = xpool.tile([128, QP], BF, tag="xg")
                nc.vector.tensor_tensor(out=xg, in0=E2[:, sl], in1=D2[:, sl],
                                        op=Alu.subtract)
                nc.scalar.activation(out=ysq[:, sl], in_=xg, func=Act.Square)
                psumR = psrp.tile([2, QP], F32, tag="psr")
                for i in range(QP // 512):
                    nc.tensor.matmul(psumR[:, i * 512:(i + 1) * 512], fold2,
                                     ysq[:, m * QP + i * 512: m * QP + (i + 1) * 512],
                                     start=True, stop=True)
                nc.scalar.activation(out=reco_sb[:, sl], in_=psumR, func=Act.Copy)
                # relayout this quarter into pixel-major rp[:, 0, :]:
                # half h, pixels p = h*8192 + m*2048 + j -> row 64h+16m+j//128
                for h in range(2):
                    src = reco_sb[h:h + 1, sl].rearrange("p (r q) -> p r q", q=WE)
                    dst = rp[64 * h + 16 * m: 64 * h + 16 * m + QP // WE, 0, :]
                    nc.sync.dma_start(out=dst, in_=src)

            # ---- segment-sum matmuls: paired pixel columns ----
            # lhsT = eq columns [2t*64, 2t*64+128) (contiguous, FWL-friendly),
            # rhs = rp[:, :, 2t:2t+2] -> psumA[128, 6]; rows 64:128 belong to
            # the odd column, cross blocks are garbage and folded away below.
            for t in range(WE // 2):
                lhsT = sub_ap(eq, 2 * t * N_SEG, [[1, 2 * N_SEG]])
                rhs = sub_ap(rp, 2 * t, [[WE, 3], [1, 2]])
                nc.tensor.matmul(psumA, lhsT, rhs, start=(t == 0),
                                 stop=(t == WE // 2 - 1))
            nc.scalar.activation(out=sbA, in_=psumA, func=Act.Copy)
            nc.tensor.matmul(psumB, foldm, sbA, start=True, stop=True)

            # ---- per-segment selection ----
            # psumB cols: reco_q, reco_q1, ones, ones, pos_q, pos_q1
            sbB = small.tile([N_SEG, 6], F32)
            nc.vector.tensor_copy(out=sbB, in_=psumB)
            nc.vector.tensor_tensor(out=err_sum, in0=sbB[:, 0:1],
                                    in1=sbB[:, 1:2], op=Alu.add)
            nc.vector.tensor_tensor(out=counts, in0=sbB[:, 2:3],
                                    in1=sbB[:, 3:4], op=Alu.add)
            nc.vector.tensor_tensor(out=poscnt, in0=sbB[:, 4:5],
                                    in1=sbB[:, 5:6], op=Alu.add)
            nc.vector.tensor_scalar(out=safe, in0=counts, scalar1=1.0, scalar2=None,
                                    op0=Alu.max)
            nc.vector.reciprocal(out=rsafe, in_=safe)
            nc.vector.scalar_tensor_tensor(out=mean_err, in0=err_sum,
                                           scalar=1.0 / C_FEAT, in1=rsafe,
                                           op0=Alu.mult, op1=Alu.mult)
            nc.vector.tensor_tensor(out=ratio, in0=poscnt, in1=rsafe, op=Alu.mult)
            thr_cnt = float(np.float32(MIN_FRAC)) * NPIX
            nc.vector.tensor_scalar(out=validt, in0=counts, scalar1=thr_cnt,
                                    scalar2=None, op0=Alu.is_ge)
            nc.vector.tensor_scalar(out=pflag, in0=ratio,
                                    scalar1=float(np.float32(MIN_FRAC)),
                                    scalar2=None, op0=Alu.is_gt)
            nc.vector.tensor_tensor(out=rhsf[0:N_SEG, 1:2], in0=validt, in1=pflag,
                                    op=Alu.mult)
            nc.vector.tensor_tensor(out=rhsf[0:N_SEG, 0:1], in0=mean_err,
                                    in1=rhsf[0:N_SEG, 1:2], op=Alu.mult)

            # ---- recovery loss elementwise (channel halves for pipelining) ----
            HP = PIX_FREE // 2
            for c in range(C_IMG):
                for h in range(2):
                    osl = sub_ap(OT, c * PIX_FREE + h * HP, [[1, HP]])
                    isl = sub_ap(IT, c * PIX_FREE + h * HP, [[1, HP]])
                    ltl = sub_ap(lt, h * HP, [[1, HP]])
                    ml = sub_ap(m01, h * HP, [[1, HP]])
                    ttile = scrp.tile([128, HP], BF, tag="t")
                    dtile = scrp.tile([128, HP], BF, tag="d")
                    dmt = scrp.tile([128, HP], BF, tag="dm")
                    sq = scrp.tile([128, HP], BF, tag="sq")
                    nc.vector.tensor_tensor(out=ttile, in0=isl, in1=ltl, op=Alu.mult)
                    nc.vector.tensor_tensor(out=dtile, in0=osl, in1=ttile,
                                            op=Alu.subtract)
                    nc.vector.tensor_tensor(out=dmt, in0=dtile, in1=ml, op=Alu.mult)
                    nc.scalar.activation(out=sq, in_=dmt, func=Act.Square,
                                         accum_out=racc[:, 2 * c + h:2 * c + h + 1])
            # sum the 6 per-chunk accumulators into rhsf[:, 2]
            nc.vector.tensor_tensor(out=racc[:, 6:7], in0=racc[:, 0:1],
                                    in1=racc[:, 1:2], op=Alu.add)
            nc.vector.tensor_tensor(out=racc[:, 7:8], in0=racc[:, 2:3],
                                    in1=racc[:, 3:4], op=Alu.add)
            nc.vector.tensor_tensor(out=racc[:, 6:7], in0=racc[:, 6:7],
                                    in1=racc[:, 4:5], op=Alu.add)
            nc.vector.tensor_tensor(out=racc[:, 7:8], in0=racc[:, 7:8],
                                    in1=racc[:, 5:6], op=Alu.add)
            nc.vector.tensor_tensor(out=rhsf[:, 2:3], in0=racc[:, 6:7],
                                    in1=racc[:, 7:8], op=Alu.add)

            # ---- final partition reduction and output ----
            nc.vector.memset(ones128, 1.0)
            nc.tensor.matmul(psumF, ones128, rhsf, start=True, stop=True)
            nc.vector.tensor_copy(out=out_sb, in_=psumF)
            nc.sync.dma_start(out=part_d, in_=out_sb)

    nc.compile()
    return nc


def _get_nc():
    if "nc" not in _CACHE:
        _CACHE["nc"] = _build()
    return _CACHE["nc"]


def _consts():
    import ml_dtypes
    iota = np.ascontiguousarray(
        np.broadcast_to(np.arange(N_SEG, dtype=np.float32), (128, N_SEG))
    ).astype(ml_dtypes.bfloat16)
    f2 = np.zeros((128, 2), dtype=np.float32)
    f2[0::2, 0] = 1.0
    f2[1::2, 1] = 1.0
    fold2 = f2.astype(ml_dtypes.bfloat16)
    fm = np.zeros((128, N_SEG), dtype=np.float32)
    fm[np.arange(64), np.arange(64)] = 1.0
    fm[64 + np.arange(64), np.arange(64)] = 1.0
    return iota, fold2, fm


def kernel(outputs, inputs, enc1, dec1, masks, segs, confidence=None,
           iteration=None, epoch=None, **_unused):
    from concourse import bass_utils

    outputs = np.asarray(outputs, dtype=np.float32)
    inputs = np.asarray(inputs, dtype=np.float32)
    enc1 = np.asarray(enc1, dtype=np.float32)
    dec1 = np.asarray(dec1, dtype=np.float32)
    masks = np.asarray(masks, dtype=np.float32)
    segs = np.asarray(segs, dtype=np.int32)

    nc = _get_nc()
    iota, fold2, foldm = _consts()
    in_maps = []
    for b in range(N_CORES):
        in_maps.append({
            "outputs": np.ascontiguousarray(outputs[b]),
            "inputs": np.ascontiguousarray(inputs[b]),
            "enc1": np.ascontiguousarray(enc1[b]),
            "dec1": np.ascontiguousarray(dec1[b]),
            "masks": np.ascontiguousarray(masks[b, 0]),
            "segs": np.ascontiguousarray(segs[b, 0]),
            "iota_row": iota,
            "fold2": fold2,
            "foldmat": foldm,
        })

    trace = bool(int(os.environ.get("KERNEL_TRACE", "0")))
    res = bass_utils.run_bass_kernel_spmd(nc, in_maps,
                                          core_ids=list(range(N_CORES)),
                                          trace=trace)
    _CACHE["last_result"] = res

    parts = np.stack([res.results[b]["partials"].reshape(4) for b in range(N_CORES)])
    f32 = np.float32
    num1 = f32(parts[:, 0].astype(np.float32).sum(dtype=np.float32))
    den1 = f32(parts[:, 1].astype(np.float32).sum(dtype=np.float32))
    num2 = f32(parts[:, 2].astype(np.float32).sum(dtype=np.float32))
    den2 = f32(parts[:, 3].astype(np.float32).sum(dtype=np.float32))
    flat_pos_mean = f32(num1 / max(den1, f32(1.0)))
    loss_recov = f32(num2 / max(den2, f32(1.0)))
    return np.asarray(f32(loss_recov + flat_pos_mean))


# revision 14
# speedup vs baseline: 1.2364x; 1.0589x over previous
"""Trainium2 Bass kernel for nn_ConfidenceLossV2 (segment_reduce, memory-bound).

Sharding: data-parallel over the batch dim — 8 batch items, one per NeuronCore.
Each core computes 4 partial scalars (segment-loss numerator/denominator and
recovery-loss numerator/denominator); the host sums them across cores and does
the two final divisions (the "psum of weighted sums and counts").

Per-core pipeline:
  - enc/dec are loaded with f32->bf16 casting SWDGE DMAs in their NATURAL
    memory order into [128, 8192] tiles where partition k=(c*2+h) holds a
    contiguous 32KB DRAM run (pixel half h of channel c) — every HBM read is
    fully contiguous.  x = enc-dec (DVE bf16 2x) and y = x^2 (ACT) in 4
    pixel-quarter chunks, pipelined against the DMA stream.
  - reco (per-pixel channel mean*64) = fold2.T @ y via PE matmuls (K=128
    contracts all 64 channels for both halves at once), psum -> SBUF, then a
    tiny SBUF->SBUF DMA relayout of the 64KB reco image into pixel-major
    rp[:, 0, :] ([128 r, 128 q] grid); rp block 1 = ones, block 2 = pos.
  - one-hot eq[p, q*64+s] = (seg[p,q]==s) via one DVE is_equal of the
    broadcast seg row against a small host-provided iota row.
  - segment sums: pixel columns PAIRED: lhsT = eq[:, 2t*64 : 2t*64+128]
    (contiguous 128 one-hot weight columns -> FWL), rhs = rp[:, :, 2t:2t+2]
    ([3 cols x 2]), accumulated into psumA [128, 6]; rows 0:64 hold column
    2t's sums, rows 64:128 column 2t+1's; a foldmat matmul adds the halves.
  - recovery loss: per channel-half t=in*lt, d=out-t, dm=d*m01 on DVE, then
    ACT Square with accum_out -> per-partition sums; lt=(mask<0.5) and
    m01=(mask>0) via tensor_scalar, the latter with accum_out giving sum(m)
    for free; pos is just lt[:, ::4]*m01[:, ::4].
  - per-segment selection on [64,1] vectors, then one ones-matmul reduces the
    4 per-partition columns to the [1,4] partials output.
"""

import os
import sys

for _p in ("/opt/trn_rl_repo",):
    if _p not in sys.path and os.path.isdir(_p):
        sys.path.insert(0, _p)

import numpy as np

N_CORES = 8
C_IMG, H, W = 3, 512, 512
C_FEAT, HE, WE = 64, 128, 128
N_SEG = 64
NPIX = HE * WE  # 16384
PIX_FREE = (H * W) // 128  # 2048 free elems per partition at image res
HALF = NPIX // 2  # 8192 pixels per half in the (c,h) layout
WALL_COT = 0.5
MIN_FRAC = 0.01
NQ = 4  # pixel-quarter chunks for the enc/dec stream
QP = HALF // NQ  # 2048 free elems per chunk

_CACHE = {}


def _build():
    import concourse.bacc as bacc
    import concourse.bass as bass
    import concourse.tile as tile
    from concourse import mybir

    dt = mybir.dt
    BF = dt.bfloat16
    F32 = dt.float32
    Alu = mybir.AluOpType
    Act = mybir.ActivationFunctionType

    nc = bacc.Bacc("TRN2", target_bir_lowering=False, debug=False,
                   enable_asserts=False, num_devices=N_CORES)

    outputs_d = nc.dram_tensor("outputs", [C_IMG, H, W], F32, kind="ExternalInput").ap()
    inputs_d = nc.dram_tensor("inputs", [C_IMG, H, W], F32, kind="ExternalInput").ap()
    enc_d = nc.dram_tensor("enc1", [C_FEAT, HE, WE], F32, kind="ExternalInput").ap()
    dec_d = nc.dram_tensor("dec1", [C_FEAT, HE, WE], F32, kind="ExternalInput").ap()
    masks_d = nc.dram_tensor("masks", [H, W], F32, kind="ExternalInput").ap()
    segs_d = nc.dram_tensor("segs", [H, W], dt.int32, kind="ExternalInput").ap()
    iota_d = nc.dram_tensor("iota_row", [128, N_SEG], dt.bfloat16,
                            kind="ExternalInput").ap()
    fold2_d = nc.dram_tensor("fold2", [128, 2], dt.bfloat16,
                             kind="ExternalInput").ap()
    foldm_d = nc.dram_tensor("foldmat", [128, N_SEG], F32,
                             kind="ExternalInput").ap()
    part_d = nc.dram_tensor("partials", [1, 4], F32, kind="ExternalOutput").ap()

    def sub_ap(t, extra_off, dims):
        # manual AP view of a tile: dims = [[step, count], ...] free dims
        return bass.AP(tensor=t.tensor, offset=t.offset + extra_off,
                       ap=[list(t.ap[0])] + [list(d) for d in dims])

    # enc/dec natural-order view: partition k = c*2 + h (contiguous 8192 f32
    # per partition), free = pixel-in-half; sliced into NQ quarter chunks.
    def natview(x, m):
        flat = x.rearrange("c (hh rr) q -> (c hh) (rr q)", hh=2)
        return flat[:, m * QP:(m + 1) * QP]

    with tile.TileContext(nc) as tc:
        with (
            tc.tile_pool(name="big", bufs=1) as big,
            tc.tile_pool(name="xpool", bufs=3) as xpool,
            tc.tile_pool(name="scr", bufs=3) as scrp,
            tc.tile_pool(name="small", bufs=1) as small,
            tc.tile_pool(name="ps", bufs=1, space="PSUM") as psp,
            tc.tile_pool(name="psr", bufs=2, space="PSUM") as psrp,
        ):
            # ---- tiles ----
            seg_rows = big.tile([128, W], dt.int32)        # every 4th image row
            segf = small.tile([128, WE], BF)               # seg ids at feature res
            iota_row = small.tile([128, N_SEG], BF)        # 0..63 per partition
            fold2 = small.tile([128, 2], BF)               # channel-fold ones
            foldm = small.tile([128, N_SEG], F32)          # [I64; I64]
            eq = big.tile([128, WE, N_SEG], BF)            # one-hot, f = q*64+s
            E2 = big.tile([128, HALF], BF)                 # enc, (c,h) layout
            D2 = big.tile([128, HALF], BF)
            ysq = big.tile([128, HALF], BF)                # (enc-dec)^2
            reco_sb = small.tile([2, HALF], BF)            # per-pixel chan sums
            rp = big.tile([128, 3, WE], BF)                # reco | ones | pos
            Mt = big.tile([128, PIX_FREE], BF)
            OT = big.tile([128, C_IMG, PIX_FREE], BF)
            IT = big.tile([128, C_IMG, PIX_FREE], BF)
            lt = big.tile([128, PIX_FREE], BF)
            m01 = big.tile([128, PIX_FREE], BF)
            racc = small.tile([128, 8], F32)
            rhsf = small.tile([128, 4], F32)
            ones128 = small.tile([128, 1], F32)
            sbA = small.tile([128, 6], F32)
            out_sb = small.tile([1, 4], F32)

            err_sum = small.tile([N_SEG, 1], F32)
            counts = small.tile([N_SEG, 1], F32)
            poscnt = small.tile([N_SEG, 1], F32)
            safe = small.tile([N_SEG, 1], F32)
            rsafe = small.tile([N_SEG, 1], F32)
            mean_err = small.tile([N_SEG, 1], F32)
            ratio = small.tile([N_SEG, 1], F32)
            validt = small.tile([N_SEG, 1], F32)
            pflag = small.tile([N_SEG, 1], F32)

            psumA = psp.tile([128, 6], F32)
            psumB = psp.tile([N_SEG, 6], F32)
            psumF = psp.tile([1, 4], F32)

            # ---- DMAs ----
            # cheap deps on the sync queue
            nc.sync.dma_start(out=seg_rows,
                              in_=segs_d.rearrange("(p r) w -> p r w", r=4)[:, 0, :])
            nc.sync.dma_start(out=iota_row, in_=iota_d)
            nc.sync.dma_start(out=fold2, in_=fold2_d)
            nc.sync.dma_start(out=foldm, in_=foldm_d)
            # SWDGE (casting) stream: masks, then enc/dec interleaved by pixel
            # quarter, then outputs/inputs per channel.  One queue = explicit
            # priority order; every HBM read is contiguous.
            nc.gpsimd.dma_start(out=Mt, in_=masks_d.rearrange("(p r) w -> p (r w)", r=4))
            for m in range(NQ):
                nc.gpsimd.dma_start(out=E2[:, m * QP:(m + 1) * QP],
                                    in_=natview(enc_d, m))
                nc.gpsimd.dma_start(out=D2[:, m * QP:(m + 1) * QP],
                                    in_=natview(dec_d, m))
            for c in range(C_IMG):
                nc.gpsimd.dma_start(
                    out=OT[:, c, :],
                    in_=outputs_d[c].rearrange("(p r) w -> p (r w)", r=4))
                nc.gpsimd.dma_start(
                    out=IT[:, c, :],
                    in_=inputs_d[c].rearrange("(p r) w -> p (r w)", r=4))

            # ---- one-hot build ----
            # segf = bf16(seg_rows[:, ::4])
            nc.vector.tensor_copy(out=segf, in_=sub_ap(seg_rows, 0, [[4, WE]]))
            # eq[p, q, s] = (segf[p, q] == iota[s])
            segf_b = sub_ap(segf, 0, [[1, WE], [0, N_SEG]])
            iota_b = sub_ap(iota_row, 0, [[0, WE], [1, N_SEG]])
            nc.vector.tensor_tensor(out=eq, in0=segf_b, in1=iota_b, op=Alu.is_equal)

            # ---- mask-derived tensors ----
            nc.vector.memset(rhsf, 0.0)
            nc.vector.tensor_scalar(out=lt, in0=Mt, scalar1=WALL_COT, scalar2=None,
                                    op0=Alu.is_lt)
            # op1 is the accumulation op when accum_out is given
            nc.vector.tensor_scalar(out=m01, in0=Mt, scalar1=0.0, scalar2=None,
                                    op0=Alu.is_gt, op1=Alu.add,
                                    accum_out=rhsf[:, 3:4])
            nc.vector.memset(rp[:, 1, :], 1.0)
            # pos = lt[:, ::4] * m01[:, ::4]  (= (mask_i<0.5)&(mask_i>0))
            nc.vector.tensor_tensor(out=rp[:, 2, :],
                                    in0=sub_ap(lt, 0, [[4, WE]]),
                                    in1=sub_ap(m01, 0, [[4, WE]]), op=Alu.mult)

            # ---- enc/dec -> squares -> reco (chunked against the DMA) ----
            for m in range(NQ):
                sl = slice(m * QP, (m + 1) * QP)
                xg = xpool.tile([128, QP], BF, tag="xg")
                nc.vector.tensor_tensor(out=xg, in0=E2[:, sl], in1=D2[:, sl],
                                        op=Alu.subtract)
                nc.scalar.activation(out=ysq[:, sl], in_=xg, func=Act.Square)
                for half in range(2):
                    e = 2 * m + half          # eighth index 0..7
                    base = e * (QP // 2)
                    psumR = psrp.tile([2, QP // 2], F32, tag="psr")
                    for i in range(2):
                        o = base + i * 512
                        nc.tensor.matmul(psumR[:, i * 512:(i + 1) * 512], fold2,
                                         ysq[:, o:o + 512], start=True, stop=True)
                    nc.scalar.activation(out=reco_sb[:, base:base + QP // 2],
                                         in_=psumR, func=Act.Copy)
                    # relayout this eighth into pixel-major rp[:, 0, :]:
                    # half h, pixel p = h*8192 + e*1024 + j -> row 64h+8e+j//128
                    for h in range(2):
                        src = reco_sb[h:h + 1, base:base + QP // 2] \
                            .rearrange("p (r q) -> p r q", q=WE)
                        dst = rp[64 * h + 8 * e: 64 * h + 8 * e + 8, 0, :]
                        nc.sync.dma_start(out=dst, in_=src)

            # ---- segment-sum matmuls: paired pixel columns ----
            # lhsT = eq columns [2t*64, 2t*64+128) (contiguous, FWL-friendly),
            # rhs = rp[:, :, 2t:2t+2] -> psumA[128, 6]; rows 64:128 belong to
            # the odd column, cross blocks are garbage and folded away below.
            for t in range(WE // 2):
                lhsT = sub_ap(eq, 2 * t * N_SEG, [[1, 2 * N_SEG]])
                rhs = sub_ap(rp, 2 * t, [[WE, 3], [1, 2]])
                nc.tensor.matmul(psumA, lhsT, rhs, start=(t == 0),
                                 stop=(t == WE // 2 - 1))
            nc.scalar.activation(out=sbA, in_=psumA, func=Act.Copy)
            nc.tensor.matmul(psumB, foldm, sbA, start=True, stop=True)

            # ---- per-segment selection ----
            # psumB cols: reco_q, reco_q1, ones, ones, pos_q, pos_q1
            sbB = small.tile([N_SEG, 6], F32)
            nc.vector.tensor_copy(out=sbB, in_=psumB)
            nc.vector.tensor_tensor(out=err_sum, in0=sbB[:, 0:1],
                                    in1=sbB[:, 1:2], op=Alu.add)
            nc.vector.tensor_tensor(out=counts, in0=sbB[:, 2:3],
                                    in1=sbB[:, 3:4], op=Alu.add)
            nc.vector.tensor_tensor(out=poscnt, in0=sbB[:, 4:5],
                                    in1=sbB[:, 5:6], op=Alu.add)
            nc.vector.tensor_scalar(out=safe, in0=counts, scalar1=1.0, scalar2=None,
                                    op0=Alu.max)
            nc.vector.reciprocal(out=rsafe, in_=safe)
            nc.vector.scalar_tensor_tensor(out=mean_err, in0=err_sum,
                                           scalar=1.0 / C_FEAT, in1=rsafe,
                                           op0=Alu.mult, op1=Alu.mult)
            nc.vector.tensor_tensor(out=ratio, in0=poscnt, in1=rsafe, op=Alu.mult)
            thr_cnt = float(np.float32(MIN_FRAC)) * NPIX
            nc.vector.tensor_scalar(out=validt, in0=counts, scalar1=thr_cnt,
                                    scalar2=None, op0=Alu.is_ge)
            nc.vector.tensor_scalar(out=pflag, in0=ratio,
                                    scalar1=float(np.float32(MIN_FRAC)),
                                    scalar2=None, op0=Alu.is_gt)
            nc.vector.tensor_tensor(out=rhsf[0:N_SEG, 1:2], in0=validt, in1=pflag,
                                    op=Alu.mult)
            nc.vector.tensor_tensor(out=rhsf[0:N_SEG, 0:1], in0=mean_err,
                                    in1=rhsf[0:N_SEG, 1:2], op=Alu.mult)

            # ---- recovery loss elementwise (channel halves for pipelining) ----
            HP = PIX_FREE // 2
            for c in range(C_IMG):
                for h in range(2):
                    osl = sub_ap(OT, c * PIX_FREE + h * HP, [[1, HP]])
                    isl = sub_ap(IT, c * PIX_FREE + h * HP, [[1, HP]])
                    ltl = sub_ap(lt, h * HP, [[1, HP]])
                    ml = sub_ap(m01, h * HP, [[1, HP]])
                    ttile = scrp.tile([128, HP], BF, tag="t")
                    dtile = scrp.tile([128, HP], BF, tag="d")
                    dmt = scrp.tile([128, HP], BF, tag="dm")
                    sq = scrp.tile([128, HP], BF, tag="sq")
                    nc.vector.tensor_tensor(out=ttile, in0=isl, in1=ltl, op=Alu.mult)
                    nc.vector.tensor_tensor(out=dtile, in0=osl, in1=ttile,
                                            op=Alu.subtract)
                    nc.vector.tensor_tensor(out=dmt, in0=dtile, in1=ml, op=Alu.mult)
                    nc.scalar.activation(out=sq, in_=dmt, func=Act.Square,
                                         accum_out=racc[:, 2 * c + h:2 * c + h + 1])
            # sum the 6 per-chunk accumulators into rhsf[:, 2]
            nc.vector.tensor_tensor(out=racc[:, 6:7], in0=racc[:, 0:1],
                                    in1=racc[:, 1:2], op=Alu.add)
            nc.vector.tensor_tensor(out=racc[:, 7:8], in0=racc[:, 2:3],
                                    in1=racc[:, 3:4], op=Alu.add)
            nc.vector.tensor_tensor(out=racc[:, 6:7], in0=racc[:, 6:7],
                                    in1=racc[:, 4:5], op=Alu.add)
            nc.vector.tensor_tensor(out=racc[:, 7:8], in0=racc[:, 7:8],
                                    in1=racc[:, 5:6], op=Alu.add)
            nc.vector.tensor_tensor(out=rhsf[:, 2:3], in0=racc[:, 6:7],
                                    in1=racc[:, 7:8], op=Alu.add)

            # ---- final partition reduction and output ----
            nc.vector.memset(ones128, 1.0)
            nc.tensor.matmul(psumF, ones128, rhsf, start=True, stop=True)
            nc.vector.tensor_copy(out=out_sb, in_=psumF)
            nc.sync.dma_start(out=part_d, in_=out_sb)

    nc.compile()
    return nc


def _get_nc():
    if "nc" not in _CACHE:
        _CACHE["nc"] = _build()
    return _CACHE["nc"]


def _consts():
    import ml_dtypes
    iota = np.ascontiguousarray(
        np.broadcast_to(np.arange(N_SEG, dtype=np.float32), (128, N_SEG))
    ).astype(ml_dtypes.bfloat16)
    f2 = np.zeros((128, 2), dtype=np.float32)
    f2[0::2, 0] = 1.0
    f2[1::2, 1] = 1.0
    fold2 = f2.astype(ml_dtypes.bfloat16)
    fm = np.zeros((128, N_SEG), dtype=np.float32)
    fm[np.arange(64), np.arange(64)] = 1.0
    fm[64 + np.arange(64), np.arange(64)] = 1.0
    return iota, fold2, fm


def kernel(outputs, inputs, enc1, dec1, masks, segs, confidence=None,
           iteration=None, epoch=None, **_unused):
    from concourse import bass_utils

    outputs = np.asarray(outputs, dtype=np.float32)
    inputs = np.asarray(inputs, dtype=np.float32)
    enc1 = np.asarray(enc1, dtype=np.float32)
    dec1 = np.asarray(dec1, dtype=np.float32)
    masks = np.asarray(masks, dtype=np.float32)
    segs = np.asarray(segs, dtype=np.int32)

    nc = _get_nc()
    iota, fold2, foldm = _consts()
    in_maps = []
    for b in range(N_CORES):
        in_maps.append({
            "outputs": np.ascontiguousarray(outputs[b]),
            "inputs": np.ascontiguousarray(inputs[b]),
            "enc1": np.ascontiguousarray(enc1[b]),
            "dec1": np.ascontiguousarray(dec1[b]),
            "masks": np.ascontiguousarray(masks[b, 0]),
            "segs": np.ascontiguousarray(segs[b, 0]),
            "iota_row": iota,
            "fold2": fold2,
            "foldmat": foldm,
        })

    trace = bool(int(os.environ.get("KERNEL_TRACE", "0")))
    res = bass_utils.run_bass_kernel_spmd(nc, in_maps,
                                          core_ids=list(range(N_CORES)),
                                          trace=trace)
    _CACHE["last_result"] = res

    parts = np.stack([res.results[b]["partials"].reshape(4) for b in range(N_CORES)])
    f32 = np.float32
    num1 = f32(parts[:, 0].astype(np.float32).sum(dtype=np.float32))
    den1 = f32(parts[:, 1].astype(np.float32).sum(dtype=np.float32))
    num2 = f32(parts[:, 2].astype(np.float32).sum(dtype=np.float32))
    den2 = f32(parts[:, 3].astype(np.float32).sum(dtype=np.float32))
    flat_pos_mean = f32(num1 / max(den1, f32(1.0)))
    loss_recov = f32(num2 / max(den2, f32(1.0)))
    return np.asarray(f32(loss_recov + flat_pos_mean))
